# revision 3
# baseline (speedup 1.0000x reference)
"""ConvFFN block kernel for 8 Trainium2 NeuronCores (v3).

Per-core dataflow (1 batch per core, full n=1024 tiles):

Phase A (stage 1), loop over 64 d-pairs p:
  - L1a: pair-block-diag 128x128 fp8 matmul (weight-stationary, 2 n-halves)
    -> pa PSUM [128=(dl,o), 1024] f32
  - gelu1 on ScalarE: ACT Gelu, bias=b1a_t[:,p], [128,1024] PSUM->SBUF fp8
  - L1b: pair matmul (w1b cols ordered (o,dl)) -> pb PSUM [128=(o,dl), 1024]
  - evac on DVE: tensor_copy pb -> H2[:, p, :] fp8  (H2[o*2+dl, p, n])

Flip (stage1->stage2 transpose) via SBUF->SBUF DMA row-gathers:
  V[m][dl*64+p, n] = H2[2m+dl, p, n]   (2 DMAs per m, 64KB each)

Phase B (stage 2), loop over 64 m:
  - L2a: dense 128x128 fp8 matmul (rows = permuted d) -> p2 PSUM
  - gelu2 on ScalarE: ACT Gelu, bias=b2a_eff[:,m] -> g2 fp8
  - L2b: dense matmul -> p3 PSUM [128=d'', 1024]
  - evac on DVE: tensor_copy p3 -> out tile fp8; DMA out per 4 m's

Host: residual add + b2b bias in f32 (out = x + b2b + g), un-permute.
"""

import sys

for _p in ("/opt/trn_rl_repo", "/opt/pypackages"):
    if _p not in sys.path:
        sys.path.append(_p)

import numpy as np
import ml_dtypes

from concourse import bacc, tile, mybir
from concourse.bass_utils import run_bass_kernel_spmd

FP8 = mybir.dt.float8e4
F32 = mybir.dt.float32
AF = mybir.ActivationFunctionType

B, D, M, N = 8, 128, 64, 1024
PAIRS = 64

_CACHE = {}


def _build_module():
    nc = bacc.Bacc("TRN2", target_bir_lowering=False, debug=False, num_devices=8)

    xb_d = nc.dram_tensor("xb", [PAIRS * 128, N], FP8, kind="ExternalInput").ap()
    w1a_d = nc.dram_tensor("w1a", [128, PAIRS, 128], FP8, kind="ExternalInput").ap()
    w1b_d = nc.dram_tensor("w1b", [128, PAIRS, 128], FP8, kind="ExternalInput").ap()
    w2a_d = nc.dram_tensor("w2a", [128, M, 128], FP8, kind="ExternalInput").ap()
    w2b_d = nc.dram_tensor("w2b", [128, M, 128], FP8, kind="ExternalInput").ap()
    b1a_d = nc.dram_tensor("b1a_t", [128, PAIRS], F32, kind="ExternalInput").ap()
    b2a_d = nc.dram_tensor("b2a_t", [128, M], F32, kind="ExternalInput").ap()
    out_d = nc.dram_tensor("out", [M * 128, N], FP8, kind="ExternalOutput").ap()

    with tile.TileContext(nc) as tc:
        with (
            tc.tile_pool(name="wpool", bufs=1) as wpool,
            tc.tile_pool(name="h2p", bufs=1) as h2p,
            tc.tile_pool(name="xbp", bufs=2) as xbp,
            tc.tile_pool(name="gp", bufs=3) as gp,
            tc.tile_pool(name="vp", bufs=6) as vp,
            tc.tile_pool(name="outp", bufs=2) as outp,
            tc.tile_pool(name="psp", bufs=1, space="PSUM") as psp,
        ):
            w1a_s = wpool.tile([128, PAIRS * 128], FP8)
            w1b_s = wpool.tile([128, PAIRS * 128], FP8)
            w2a_s = wpool.tile([128, M * 128], FP8)
            w2b_s = wpool.tile([128, M * 128], FP8)
            b1a_s = wpool.tile([128, PAIRS], F32)
            b2a_s = wpool.tile([128, M], F32)

            for t, d in ((w1a_s, w1a_d), (w1b_s, w1b_d)):
                nc.sync.dma_start(
                    out=t[:].rearrange("k (p j) -> k p j", j=128), in_=d[:]
                )
            nc.sync.dma_start(out=b1a_s[:], in_=b1a_d[:])
            for t, d in ((w2a_s, w2a_d), (w2b_s, w2b_d)):
                nc.sync.dma_start(
                    out=t[:].rearrange("k (p j) -> k p j", j=128), in_=d[:]
                )
            nc.sync.dma_start(out=b2a_s[:], in_=b2a_d[:])

            h2 = h2p.tile([128, PAIRS, N], FP8, tag="H2", name="h2")

            # ---- phase A: stage 1, d-major, weight-stationary ----
            xb_t = None
            for p in range(PAIRS):
                if p % 4 == 0:
                    xb_t = xbp.tile([128, 4, N], FP8, tag="xb", name=f"xb{p}")
                    nc.gpsimd.dma_start(
                        out=xb_t[:],
                        in_=xb_d[p * 128:(p + 4) * 128, :]
                        .rearrange("(pp q) t -> q pp t", q=128),
                    )
                pa = psp.tile([128, N], F32, tag=("psA", "psB")[p % 2],
                              bufs=1, name=f"pa{p}")
                for h in range(2):
                    nc.tensor.matmul(
                        pa[:, h * 512:(h + 1) * 512],
                        w1a_s[:, p * 128:(p + 1) * 128],
                        xb_t[:, p % 4, h * 512:(h + 1) * 512],
                        start=True, stop=True,
                    )
                g1 = gp.tile([128, N], FP8, tag="g1", name=f"g1_{p}")
                nc.scalar.activation(
                    g1[:], pa[:], AF.Gelu, bias=b1a_s[:, p:p + 1], scale=1.0
                )
                pb = psp.tile([128, N], F32, tag=("psC", "psD")[p % 2],
                              bufs=1, name=f"pb{p}")
                for h in range(2):
                    nc.tensor.matmul(
                        pb[:, h * 512:(h + 1) * 512],
                        w1b_s[:, p * 128:(p + 1) * 128],
                        g1[:, h * 512:(h + 1) * 512],
                        start=True, stop=True,
                    )
                if p % 21 == 20:
                    nc.scalar.activation(h2[:, p, :], pb[:], AF.Copy)
                else:
                    nc.vector.tensor_copy(h2[:, p, :], pb[:])

            # ---- flip + phase B: stage 2, m-major, weight-stationary ----
            # V-gather: one DMA per m; V[dl*64+q, n] = H2[2m+dl, q, n]
            def v_gather(m):
                v_t = vp.tile([128, N], FP8, tag="V", name=f"v{m}")
                for dl in range(2):
                    nc.gpsimd.dma_start(
                        out=v_t[dl * 64:(dl + 1) * 64, :],
                        in_=h2[2 * m + dl:2 * m + dl + 1, :, :],
                    )
                return v_t

            out_t = None
            for m in range(M):
                v_t = v_gather(m)
                p2 = psp.tile([128, N], F32, tag=("psA", "psB")[m % 2],
                              bufs=1, name=f"p2_{m}")
                for h in range(2):
                    nc.tensor.matmul(
                        p2[:, h * 512:(h + 1) * 512],
                        w2a_s[:, m * 128:(m + 1) * 128],
                        v_t[:, h * 512:(h + 1) * 512],
                        start=True, stop=True,
                    )
                g2 = gp.tile([128, N], FP8, tag="g2", name=f"g2_{m}")
                nc.scalar.activation(
                    g2[:], p2[:], AF.Gelu, bias=b2a_s[:, m:m + 1], scale=1.0
                )
                p3 = psp.tile([128, N], F32, tag=("psC", "psD")[m % 2],
                              bufs=1, name=f"p3_{m}")
                for h in range(2):
                    nc.tensor.matmul(
                        p3[:, h * 512:(h + 1) * 512],
                        w2b_s[:, m * 128:(m + 1) * 128],
                        g2[:, h * 512:(h + 1) * 512],
                        start=True, stop=True,
                    )
                if m % 4 == 0:
                    out_t = outp.tile([128, 4, N], FP8, tag="osb",
                                      name=f"osb{m // 4}")
                if m % 21 == 20:
                    nc.scalar.activation(out_t[:, m % 4, :], p3[:], AF.Copy)
                else:
                    nc.vector.tensor_copy(out_t[:, m % 4, :], p3[:])
                if m % 4 == 3:
                    m0 = m - 3
                    nc.gpsimd.dma_start(
                        out=out_d[m0 * 128:(m0 + 4) * 128, :]
                        .rearrange("(mi k) t -> k mi t", k=128),
                        in_=out_t[:],
                    )

    nc.compile()
    return nc


def _host_prep(x, W1a, b1a, W1b, b1b, W2a, b2a, W2b, b2b):
    f8 = ml_dtypes.float8_e4m3

    # xb rows (p, dl, m): d = 2p + dl
    xq = x.astype(f8)  # (B, 128, 64, 1024)
    xb = np.ascontiguousarray(xq.reshape(B, PAIRS * 128, N))

    # w1a: rows (dl, i) -> cols (dl', o), block-diagonal per pair
    Wa = W1a.reshape(PAIRS, 2, M, M)  # (p, dl, o, i)
    A4 = np.zeros((2, M, PAIRS, 2, M), np.float32)  # (dl, i, p, dl', o)
    A4[0, :, :, 0, :] = Wa[:, 0].transpose(2, 0, 1)
    A4[1, :, :, 1, :] = Wa[:, 1].transpose(2, 0, 1)
    w1a = np.ascontiguousarray(A4.reshape(128, PAIRS, 128)).astype(f8)

    # w1b: rows (dl, i) -> cols (o, dl')
    Wb = W1b.reshape(PAIRS, 2, M, M)
    B4 = np.zeros((2, M, PAIRS, M, 2), np.float32)  # (dl, i, p, o, dl')
    B4[0, :, :, :, 0] = Wb[:, 0].transpose(2, 0, 1)
    B4[1, :, :, :, 1] = Wb[:, 1].transpose(2, 0, 1)
    w1b = np.ascontiguousarray(B4.reshape(128, PAIRS, 128)).astype(f8)

    # b1a_t: partition (dl, o), col p: b1a[2p+dl, o]
    b1a_t = np.ascontiguousarray(
        b1a.reshape(PAIRS, 2, M).transpose(1, 2, 0).reshape(128, PAIRS)
    ).astype(np.float32)

    # V partition q <-> d(q) = 2*(q%64) + (q//64)
    q = np.arange(128)
    dq = 2 * (q % 64) + (q // 64)
    # w2a rows q = V-order d, cols o' natural
    w2a = np.ascontiguousarray(
        W2a.transpose(2, 0, 1)[dq]  # (i=d -> q, m, o')
    ).astype(f8)
    w2b = np.ascontiguousarray(W2b.transpose(2, 0, 1)).astype(f8)

    b2a_eff = b2a + np.einsum("moi,im->mo", W2a, b1b)
    b2a_t = np.ascontiguousarray(b2a_eff.T).astype(np.float32)

    shared = {
        "w1a": w1a, "w1b": w1b, "w2a": w2a, "w2b": w2b,
        "b1a_t": b1a_t, "b2a_t": b2a_t,
    }
    return [{"xb": np.ascontiguousarray(xb[b]), **shared} for b in range(B)]


def kernel(x, W1a, b1a, W1b, b1b, W2a, b2a, W2b, b2b, _trace=False, _tmpdir=None):
    x, W1a, b1a, W1b, b1b, W2a, b2a, W2b, b2b = (
        np.asarray(a, dtype=np.float32)
        for a in (x, W1a, b1a, W1b, b1b, W2a, b2a, W2b, b2b)
    )
    if "nc" not in _CACHE:
        _CACHE["nc"] = _build_module()
    nc = _CACHE["nc"]
    in_maps = _host_prep(x, W1a, b1a, W1b, b1b, W2a, b2a, W2b, b2b)
    res = run_bass_kernel_spmd(
        nc, in_maps, list(range(8)), trace=_trace, tmpdir=_tmpdir
    )
    _CACHE["last_result"] = res
    out = np.stack(
        [np.asarray(res.results[b]["out"]).astype(np.float32) for b in range(B)]
    )
    # rows (m, d''), cols n -> (b, d, m, n); host residual + b2b
    g = out.reshape(B, M, 128, N).transpose(0, 2, 1, 3)
    return np.ascontiguousarray(
        x + b2b.T[None, :, :, None] + g
    ).astype(np.float32)


# revision 4
# speedup vs baseline: 1.5268x; 1.5268x over previous
"""ConvFFN block kernel for 8 Trainium2 NeuronCores (v3).

Per-core dataflow (1 batch per core, full n=1024 tiles):

Phase A (stage 1), loop over 64 d-pairs p:
  - L1a: pair-block-diag 128x128 fp8 matmul (weight-stationary, 2 n-halves)
    -> pa PSUM [128=(dl,o), 1024] f32
  - gelu1 on ScalarE: ACT Gelu, bias=b1a_t[:,p], [128,1024] PSUM->SBUF fp8
  - L1b: pair matmul (w1b cols ordered (o,dl)) -> pb PSUM [128=(o,dl), 1024]
  - evac on DVE: tensor_copy pb -> H2[:, p, :] fp8  (H2[o*2+dl, p, n])

Flip (stage1->stage2 transpose) via SBUF->SBUF DMA row-gathers:
  V[m][dl*64+p, n] = H2[2m+dl, p, n]   (2 DMAs per m, 64KB each)

Phase B (stage 2), loop over 64 m:
  - L2a: dense 128x128 fp8 matmul (rows = permuted d) -> p2 PSUM
  - gelu2 on ScalarE: ACT Gelu, bias=b2a_eff[:,m] -> g2 fp8
  - L2b: dense matmul -> p3 PSUM [128=d'', 1024]
  - evac on DVE: tensor_copy p3 -> out tile fp8; DMA out per 4 m's

Host: residual add + b2b bias in f32 (out = x + b2b + g), un-permute.
"""

import sys

for _p in ("/opt/trn_rl_repo", "/opt/pypackages"):
    if _p not in sys.path:
        sys.path.append(_p)

import numpy as np
import ml_dtypes

from concourse import bacc, tile, mybir
from concourse.bass_utils import run_bass_kernel_spmd

FP8 = mybir.dt.float8e4
F32 = mybir.dt.float32
AF = mybir.ActivationFunctionType

B, D, M, N = 8, 128, 64, 1024
PAIRS = 64

_CACHE = {}


def _build_module():
    nc = bacc.Bacc("TRN2", target_bir_lowering=False, debug=False, num_devices=8)

    xb_d = nc.dram_tensor("xb", [PAIRS * 128, N], FP8, kind="ExternalInput").ap()
    w1a_d = nc.dram_tensor("w1a", [128, PAIRS, 128], FP8, kind="ExternalInput").ap()
    w1b_d = nc.dram_tensor("w1b", [128, PAIRS, 128], FP8, kind="ExternalInput").ap()
    w2a_d = nc.dram_tensor("w2a", [128, M, 128], FP8, kind="ExternalInput").ap()
    w2b_d = nc.dram_tensor("w2b", [128, M, 128], FP8, kind="ExternalInput").ap()
    b1a_d = nc.dram_tensor("b1a_t", [128, PAIRS], F32, kind="ExternalInput").ap()
    b2a_d = nc.dram_tensor("b2a_t", [128, M], F32, kind="ExternalInput").ap()
    out_d = nc.dram_tensor("out", [M * 128, N], FP8, kind="ExternalOutput").ap()

    with tile.TileContext(nc) as tc:
        with (
            tc.tile_pool(name="wpool", bufs=1) as wpool,
            tc.tile_pool(name="h2p", bufs=2) as h2p,
            tc.tile_pool(name="dramp", bufs=1, space="DRAM") as dramp,
            tc.tile_pool(name="xbp", bufs=2) as xbp,
            tc.tile_pool(name="gp", bufs=3) as gp,
            tc.tile_pool(name="vp", bufs=4) as vp,
            tc.tile_pool(name="outp", bufs=2) as outp,
            tc.tile_pool(name="psp", bufs=1, space="PSUM") as psp,
        ):
            w1a_s = wpool.tile([128, PAIRS * 128], FP8)
            w1b_s = wpool.tile([128, PAIRS * 128], FP8)
            w2a_s = wpool.tile([128, M * 128], FP8)
            w2b_s = wpool.tile([128, M * 128], FP8)
            b1a_s = wpool.tile([128, PAIRS], F32)
            b2a_s = wpool.tile([128, M], F32)

            for t, d in ((w1a_s, w1a_d), (w1b_s, w1b_d)):
                nc.sync.dma_start(
                    out=t[:].rearrange("k (p j) -> k p j", j=128), in_=d[:]
                )
            nc.sync.dma_start(out=b1a_s[:], in_=b1a_d[:])
            for t, d in ((w2a_s, w2a_d), (w2b_s, w2b_d)):
                nc.sync.dma_start(
                    out=t[:].rearrange("k (p j) -> k p j", j=128), in_=d[:]
                )
            nc.sync.dma_start(out=b2a_s[:], in_=b2a_d[:])

            h2d = dramp.tile([128, PAIRS, N], FP8, tag="H2D", name="h2d")

            # ---- phase A: stage 1, d-major, weight-stationary ----
            xb_t = None
            h2b = None
            for p in range(PAIRS):
                if p % 8 == 0:
                    h2b = h2p.tile([128, 8, N], FP8, tag="H2B",
                                   name=f"h2b{p // 8}")
                if p % 4 == 0:
                    xb_t = xbp.tile([128, 4, N], FP8, tag="xb", name=f"xb{p}")
                    nc.gpsimd.dma_start(
                        out=xb_t[:],
                        in_=xb_d[p * 128:(p + 4) * 128, :]
                        .rearrange("(pp q) t -> q pp t", q=128),
                    )
                pa = psp.tile([128, N], F32, tag=("psA", "psB")[p % 2],
                              bufs=1, name=f"pa{p}")
                for h in range(2):
                    nc.tensor.matmul(
                        pa[:, h * 512:(h + 1) * 512],
                        w1a_s[:, p * 128:(p + 1) * 128],
                        xb_t[:, p % 4, h * 512:(h + 1) * 512],
                        start=True, stop=True,
                    )
                g1 = gp.tile([128, N], FP8, tag="g1", name=f"g1_{p}")
                nc.scalar.activation(
                    g1[:], pa[:], AF.Gelu, bias=b1a_s[:, p:p + 1], scale=1.0
                )
                pb = psp.tile([128, N], F32, tag=("psC", "psD")[p % 2],
                              bufs=1, name=f"pb{p}")
                for h in range(2):
                    nc.tensor.matmul(
                        pb[:, h * 512:(h + 1) * 512],
                        w1b_s[:, p * 128:(p + 1) * 128],
                        g1[:, h * 512:(h + 1) * 512],
                        start=True, stop=True,
                    )
                if p % 21 == 20:
                    nc.scalar.activation(h2b[:, p % 8, :], pb[:], AF.Copy)
                else:
                    nc.vector.tensor_copy(h2b[:, p % 8, :], pb[:])
                if p % 8 == 7:
                    nc.gpsimd.dma_start(
                        out=h2d[:, p - 7:p + 1, :], in_=h2b[:]
                    )

            # ---- flip + phase B: stage 2, m-major, weight-stationary ----
            # V-gather: DRAM-side rearrange; V[dl*64+q, m, n] = h2d[2m+dl, q, n]
            h2r = h2d[:].rearrange("(mm dl) q n -> dl q mm n", dl=2)

            def v_gather(mc):
                v_c = vp.tile([128, 16, N], FP8, tag="V", name=f"v{mc}")
                for dl in range(2):
                    nc.sync.dma_start(
                        out=v_c[dl * 64:(dl + 1) * 64, :, :],
                        in_=h2r[dl:dl + 1, :, mc * 16:(mc + 1) * 16, :],
                    )
                return v_c

            v_c = None
            out_t = None
            for m in range(M):
                if m % 16 == 0:
                    v_c = v_gather(m // 16)
                p2 = psp.tile([128, N], F32, tag=("psA", "psB")[m % 2],
                              bufs=1, name=f"p2_{m}")
                for h in range(2):
                    nc.tensor.matmul(
                        p2[:, h * 512:(h + 1) * 512],
                        w2a_s[:, m * 128:(m + 1) * 128],
                        v_c[:, m % 16, h * 512:(h + 1) * 512],
                        start=True, stop=True,
                    )
                g2 = gp.tile([128, N], FP8, tag="g2", name=f"g2_{m}")
                nc.scalar.activation(
                    g2[:], p2[:], AF.Gelu, bias=b2a_s[:, m:m + 1], scale=1.0
                )
                p3 = psp.tile([128, N], F32, tag=("psC", "psD")[m % 2],
                              bufs=1, name=f"p3_{m}")
                for h in range(2):
                    nc.tensor.matmul(
                        p3[:, h * 512:(h + 1) * 512],
                        w2b_s[:, m * 128:(m + 1) * 128],
                        g2[:, h * 512:(h + 1) * 512],
                        start=True, stop=True,
                    )
                if m % 4 == 0:
                    out_t = outp.tile([128, 4, N], FP8, tag="osb",
                                      name=f"osb{m // 4}")
                if m % 21 == 20:
                    nc.scalar.activation(out_t[:, m % 4, :], p3[:], AF.Copy)
                else:
                    nc.vector.tensor_copy(out_t[:, m % 4, :], p3[:])
                if m % 4 == 3:
                    m0 = m - 3
                    nc.gpsimd.dma_start(
                        out=out_d[m0 * 128:(m0 + 4) * 128, :]
                        .rearrange("(mi k) t -> k mi t", k=128),
                        in_=out_t[:],
                    )

    nc.compile()
    return nc


def _host_prep(x, W1a, b1a, W1b, b1b, W2a, b2a, W2b, b2b):
    f8 = ml_dtypes.float8_e4m3

    # xb rows (p, dl, m): d = 2p + dl
    xq = x.astype(f8)  # (B, 128, 64, 1024)
    xb = np.ascontiguousarray(xq.reshape(B, PAIRS * 128, N))

    # w1a: rows (dl, i) -> cols (dl', o), block-diagonal per pair
    Wa = W1a.reshape(PAIRS, 2, M, M)  # (p, dl, o, i)
    A4 = np.zeros((2, M, PAIRS, 2, M), np.float32)  # (dl, i, p, dl', o)
    A4[0, :, :, 0, :] = Wa[:, 0].transpose(2, 0, 1)
    A4[1, :, :, 1, :] = Wa[:, 1].transpose(2, 0, 1)
    w1a = np.ascontiguousarray(A4.reshape(128, PAIRS, 128)).astype(f8)

    # w1b: rows (dl, i) -> cols (o, dl')
    Wb = W1b.reshape(PAIRS, 2, M, M)
    B4 = np.zeros((2, M, PAIRS, M, 2), np.float32)  # (dl, i, p, o, dl')
    B4[0, :, :, :, 0] = Wb[:, 0].transpose(2, 0, 1)
    B4[1, :, :, :, 1] = Wb[:, 1].transpose(2, 0, 1)
    w1b = np.ascontiguousarray(B4.reshape(128, PAIRS, 128)).astype(f8)

    # b1a_t: partition (dl, o), col p: b1a[2p+dl, o]
    b1a_t = np.ascontiguousarray(
        b1a.reshape(PAIRS, 2, M).transpose(1, 2, 0).reshape(128, PAIRS)
    ).astype(np.float32)

    # V partition q <-> d(q) = 2*(q%64) + (q//64)
    q = np.arange(128)
    dq = 2 * (q % 64) + (q // 64)
    # w2a rows q = V-order d, cols o' natural
    w2a = np.ascontiguousarray(
        W2a.transpose(2, 0, 1)[dq]  # (i=d -> q, m, o')
    ).astype(f8)
    w2b = np.ascontiguousarray(W2b.transpose(2, 0, 1)).astype(f8)

    b2a_eff = b2a + np.einsum("moi,im->mo", W2a, b1b)
    b2a_t = np.ascontiguousarray(b2a_eff.T).astype(np.float32)

    shared = {
        "w1a": w1a, "w1b": w1b, "w2a": w2a, "w2b": w2b,
        "b1a_t": b1a_t, "b2a_t": b2a_t,
    }
    return [{"xb": np.ascontiguousarray(xb[b]), **shared} for b in range(B)]


def kernel(x, W1a, b1a, W1b, b1b, W2a, b2a, W2b, b2b, _trace=False, _tmpdir=None):
    x, W1a, b1a, W1b, b1b, W2a, b2a, W2b, b2b = (
        np.asarray(a, dtype=np.float32)
        for a in (x, W1a, b1a, W1b, b1b, W2a, b2a, W2b, b2b)
    )
    if "nc" not in _CACHE:
        _CACHE["nc"] = _build_module()
    nc = _CACHE["nc"]
    in_maps = _host_prep(x, W1a, b1a, W1b, b1b, W2a, b2a, W2b, b2b)
    res = run_bass_kernel_spmd(
        nc, in_maps, list(range(8)), trace=_trace, tmpdir=_tmpdir
    )
    _CACHE["last_result"] = res
    out = np.stack(
        [np.asarray(res.results[b]["out"]).astype(np.float32) for b in range(B)]
    )
    # rows (m, d''), cols n -> (b, d, m, n); host residual + b2b
    g = out.reshape(B, M, 128, N).transpose(0, 2, 1, 3)
    return np.ascontiguousarray(
        x + b2b.T[None, :, :, None] + g
    ).astype(np.float32)
